# revision 9
# baseline (speedup 1.0000x reference)
"""Trainium2 Bass kernel for nn_C_Loss_52338471469527 (BCE + fd + patch loss).

Strategy (pure data-parallel over batch, 8 cores x 8 images):
  Device computes, per core, from its (8,512,512) shard of prediction/gt:
    - bce partial sums  Sum ln(sel), sel = g?p:(1-p), via prod=(1-2p)(1-2g)
      and ACT Ln with input affine ln(-0.5*prod+0.5)  [phase 2, ln table set]
    - patch-grid partials: per-pixel |sig-g| = |tanh(p/2)+(1-2g)|/2 accumulated
      per tile-row-position, column-block-64 sums on device
    - fd partials: 4x4-pooled sums of f=th+ug and ug via stride-4 accumulating
      PE matmuls; Sum|pooled| and count(pooled gt>0) accumulated on device
  Host (numpy, float64) does the final tiny nonlinear combine across cores.
"""

import os
import sys

os.environ.setdefault("MYCRO_LOCAL_CACHE", "1")
sys.path.insert(0, "/opt/trn_rl_repo")

from contextlib import ExitStack

import ml_dtypes
import numpy as np

import concourse.bass as bass
import concourse.tile as tile
from concourse import bacc, mybir
from concourse.bass_utils import run_bass_kernel_spmd

F32 = mybir.dt.float32
BF16 = mybir.dt.bfloat16
AFT = mybir.ActivationFunctionType
ALU = mybir.AluOpType

N, H, W = 64, 512, 512
NCORES = 8
NPER = N // NCORES          # images per core
ROWS = NPER * H             # 4096 rows per core
TILE_P, TILE_F = 128, 512
NT = ROWS // TILE_P         # 32 tiles per core
TPOS = H // TILE_P          # 4 tile positions per image
LN_CHUNK = 4                # tiles per phase-2 Ln op
NLN = NT // LN_CHUNK        # 8 Ln ops

BETA, ALPHA, GAMMA = 1.1, 0.03, 0.02
EPS = 1e-6
PATCH_SIZES = (256, 128, 64)


def _device_program(ctx: ExitStack, tc: "tile.TileContext", aps: dict):
    nc = tc.nc
    p_d, g_d = aps["p"], aps["g"]
    p4_d = aps["p4"]
    o_dseg, o_bce, o_fd, o_cnt = aps["o_dseg"], aps["o_bce"], aps["o_fd"], aps["o_cnt"]

    io = ctx.enter_context(tc.tile_pool(name="io", bufs=4))
    work = ctx.enter_context(tc.tile_pool(name="work", bufs=4))
    persist = ctx.enter_context(tc.tile_pool(name="persist", bufs=1))
    pe_pool = ctx.enter_context(tc.tile_pool(name="pe", bufs=4, space="PSUM"))
    uacc_pool = ctx.enter_context(tc.tile_pool(name="uacc", bufs=1, space="PSUM"))

    # Constant pooling matrix P4[k, m] = (k//4 == m)
    P4 = persist.tile([128, 32], BF16, tag="p4c")
    nc.sync.dma_start(P4[:], p4_d)

    # Persistent state
    prod_slab = persist.tile([128, NT * TILE_F], BF16, tag="prod")  # 32KB/part
    acc_d = [
        persist.tile([128, TILE_F], BF16, tag=f"accd{t}", name=f"accd{t}")
        for t in range(TPOS)
    ]
    fdabs_cols = persist.tile([32, NT], F32, tag="fdabs")
    fdcnt_cols = persist.tile([32, NT], F32, tag="fdcnt")
    bce_acc = persist.tile([128, NLN], F32, tag="bce")
    half = persist.tile([128, 1], F32, tag="half")
    uacc = [
        uacc_pool.tile([32, 128], F32, tag=f"uacc{t}", name=f"uacc{t}")
        for t in range(TPOS)
    ]

    for t in range(TPOS):
        nc.gpsimd.memset(acc_d[t][:], 0.0)
    nc.vector.memset(half[:], 0.5)

    # ---- Phase 1: streaming over 32 tiles (tanh table set on ACT) ----
    for i in range(NT):
        t = i % TPOS
        img = i // TPOS
        first_at_t = img == 0
        last_at_t = img == NPER - 1

        pt = io.tile([TILE_P, TILE_F], F32, tag="p")
        nc.sync.dma_start(pt[:], p_d[i * TILE_P:(i + 1) * TILE_P, :])
        gt = io.tile([TILE_P, TILE_F], F32, tag="g")
        nc.sync.dma_start(gt[:], g_d[i * TILE_P:(i + 1) * TILE_P, :])

        # th = tanh(p/2) = 2*sigmoid(p)-1   (ACT, bf16 out)
        th = work.tile([TILE_P, TILE_F], BF16, tag="th")
        nc.scalar.activation(th[:], pt[:], AFT.Tanh, bias=0.0, scale=0.5)

        # up = 1-2p (DVE ts, bf16 out); ug = 1-2g (GPSIMD ts, bf16 out)
        up = work.tile([TILE_P, TILE_F], BF16, tag="up")
        nc.vector.tensor_scalar(up[:], pt[:], -2.0, 1.0, op0=ALU.mult, op1=ALU.add)
        ug = work.tile([TILE_P, TILE_F], BF16, tag="ug")
        nc.gpsimd.tensor_scalar(ug[:], gt[:], -2.0, 1.0, op0=ALU.mult, op1=ALU.add)

        # prod = up*ug (DVE bf16 2x) -> BCE operand, stored for phase 2
        nc.vector.tensor_mul(prod_slab[:, i * TILE_F:(i + 1) * TILE_F], up[:], ug[:])

        # f = th + ug = 2*(sig - g)  (GPSIMD 2-input)
        f = work.tile([TILE_P, TILE_F], BF16, tag="f")
        nc.gpsimd.tensor_add(f[:], th[:], ug[:])

        # |f| = ug*f = ug*th + 1 (ug in {-1,1}, th>0); accumulate m = ug*th,
        # host adds the +1*NPER constant.
        m = work.tile([TILE_P, TILE_F], BF16, tag="m")
        nc.vector.tensor_mul(m[:], ug[:], th[:])
        nc.vector.tensor_add(acc_d[t][:], acc_d[t][:], m[:])

        # PE 4x4 pooling: peF = sum_j P4^T @ f[:, j::4]  (fresh per tile)
        fv = f[:].rearrange("p (n j) -> p n j", j=4)
        uv = ug[:].rearrange("p (n j) -> p n j", j=4)
        peF = pe_pool.tile([32, 128], F32, tag="pe")
        for j in range(4):
            nc.tensor.matmul(
                peF[:], P4[:], fv[:, :, j], start=(j == 0), stop=(j == 3)
            )
        peU = pe_pool.tile([32, 128], F32, tag="pe")
        for j in range(4):
            nc.tensor.matmul(
                peU[:], P4[:], uv[:, :, j], start=(j == 0), stop=(j == 3)
            )
        # uacc[t] += pooled ug  (persistent PSUM accumulation across images)
        for j in range(4):
            nc.tensor.matmul(
                uacc[t][:],
                P4[:],
                uv[:, :, j],
                start=(first_at_t and j == 0),
                stop=(last_at_t and j == 3),
                skip_group_check=True,
            )

        # fdabs_cols[:, i] = sum|peF| ; fdcnt_cols[:, i] = count(peU < 16)
        nc.vector.tensor_reduce(
            fdabs_cols[:, i:i + 1], peF[:], axis=mybir.AxisListType.X,
            op=ALU.add, apply_absolute_value=True,
        )
        cnt_scratch = work.tile([32, 128], F32, tag="cnts")
        nc.vector.tensor_scalar(
            cnt_scratch[:], peU[:], 16.0, 0.0, op0=ALU.is_lt, op1=ALU.add,
            accum_out=fdcnt_cols[:, i:i + 1],
        )

    # ---- Phase 2: BCE ln over the prod slab (ln table set on ACT) ----
    lnw = NT * TILE_F // NLN
    for k in range(NLN):
        lo = work.tile([128, lnw], BF16, tag="lnout")
        nc.scalar.activation(
            lo[:],
            prod_slab[:, k * lnw:(k + 1) * lnw],
            AFT.Ln,
            bias=half[:],
            scale=-0.5,
            accum_out=bce_acc[:, k:k + 1],
        )

    # ---- Final on-device reductions + output DMA ----
    dseg = persist.tile([128, 4 * 8], F32, tag="dseg")
    for t in range(TPOS):
        av = acc_d[t][:].rearrange("p (b c) -> p b c", c=64)
        nc.vector.tensor_reduce(
            dseg[:, t * 8:(t + 1) * 8], av, axis=mybir.AxisListType.X, op=ALU.add
        )
    fd_out = persist.tile([32, 2], F32, tag="fdout")
    nc.vector.tensor_reduce(
        fd_out[:, 0:1], fdabs_cols[:], axis=mybir.AxisListType.X, op=ALU.add
    )
    nc.vector.tensor_reduce(
        fd_out[:, 1:2], fdcnt_cols[:], axis=mybir.AxisListType.X, op=ALU.add
    )

    cnt_sb = persist.tile([32, 512], F32, tag="cntsb")
    for t in range(TPOS):
        nc.scalar.copy(cnt_sb[:, t * 128:(t + 1) * 128], uacc[t][:])

    nc.sync.dma_start(o_dseg, dseg[:])
    nc.sync.dma_start(o_bce, bce_acc[:])
    nc.sync.dma_start(o_fd, fd_out[:])
    nc.sync.dma_start(o_cnt, cnt_sb[:])


_built = {}


def _build():
    if "nc" in _built:
        return _built["nc"]
    nc = bacc.Bacc(
        "TRN2", target_bir_lowering=False, debug=False, num_devices=NCORES
    )
    aps = {
        "p": nc.dram_tensor("p", [ROWS, W], F32, kind="ExternalInput").ap(),
        "g": nc.dram_tensor("g", [ROWS, W], F32, kind="ExternalInput").ap(),
        "p4": nc.dram_tensor("p4", [128, 32], BF16, kind="ExternalInput").ap(),
        "o_dseg": nc.dram_tensor("o_dseg", [128, 32], F32, kind="ExternalOutput").ap(),
        "o_bce": nc.dram_tensor("o_bce", [128, NLN], F32, kind="ExternalOutput").ap(),
        "o_fd": nc.dram_tensor("o_fd", [32, 2], F32, kind="ExternalOutput").ap(),
        "o_cnt": nc.dram_tensor("o_cnt", [32, 512], F32, kind="ExternalOutput").ap(),
    }
    with tile.TileContext(nc) as tc:
        with ExitStack() as ctx:
            _device_program(ctx, tc, aps)
    nc.compile()
    _built["nc"] = nc
    return nc


def _p4_matrix():
    m = np.zeros((128, 32), dtype=ml_dtypes.bfloat16)
    m[np.arange(128), np.arange(128) // 4] = 1.0
    return m


def _make_in_maps(prediction: np.ndarray, ground_truth: np.ndarray):
    p = np.ascontiguousarray(
        np.asarray(prediction, dtype=np.float32).reshape(N * H, W)
    )
    g = np.ascontiguousarray(
        np.asarray(ground_truth, dtype=np.float32).reshape(N * H, W)
    )
    p4 = _p4_matrix()
    maps = []
    for c in range(NCORES):
        sl = slice(c * ROWS, (c + 1) * ROWS)
        maps.append({"p": p[sl], "g": g[sl], "p4": p4})
    return maps


def _combine(results: list) -> np.float32:
    """Host-side final combine across the 8 cores (float64)."""
    bce_sum = 0.0
    dsum8 = np.zeros((8, 8), dtype=np.float64)   # per-512x512-grid 64px blocks
    cnt8 = np.zeros((8, 8), dtype=np.float64)
    fdabs_sum = 0.0
    fdcnt_sum = 0.0
    for r in results:
        bce_sum += np.sum(r["o_bce"].astype(np.float64))
        dseg = r["o_dseg"].astype(np.float64)      # [128, 4*8]
        for t in range(TPOS):
            blk = dseg[:, t * 8:(t + 1) * 8]       # [128, 8] sums of m=ug*th
            # sum|f| over 64-row half = sum(m) + 64rows*64cols*NPER
            const = 64.0 * 64.0 * NPER
            dsum8[2 * t, :] += 0.5 * (blk[:64, :].sum(axis=0) + const)
            dsum8[2 * t + 1, :] += 0.5 * (blk[64:, :].sum(axis=0) + const)
        ocnt = r["o_cnt"].astype(np.float64)       # [32, 4*128]
        for t in range(TPOS):
            u = ocnt[:, t * 128:(t + 1) * 128]     # [32, 128] pooled-ug sums
            # u = sum_imgs sum_4x4 ug, ug = 1-2g  =>  sum_imgs sum_4x4 g:
            gc = (16.0 * NPER - u) / 2.0
            for rb in range(2):
                for cb in range(8):
                    cnt8[2 * t + rb, cb] += gc[
                        rb * 16:(rb + 1) * 16, cb * 16:(cb + 1) * 16
                    ].sum()
        ofd = r["o_fd"].astype(np.float64)
        fdabs_sum += ofd[:, 0].sum()
        fdcnt_sum += ofd[:, 1].sum()

    # BCE
    bce = -bce_sum / (N * H * W)

    # Patch loss
    total = 0.0
    for ps in PATCH_SIZES:
        k = ps // 64                      # 64px blocks per patch side
        nh = 8 // k
        d = dsum8.reshape(nh, k, nh, k).sum(axis=(1, 3))
        c = cnt8.reshape(nh, k, nh, k).sum(axis=(1, 3))
        cc = d / np.maximum(c, EPS)
        total_sum = np.sum(float(ps * N) * cc ** 2)
        local = np.clip(np.sqrt(total_sum) / 512.0 / N, 0.0, 1.0)
        total += local
    loss_patch = total / 64.0

    # fd loss
    fd_abs = fdabs_sum / 32.0             # sum |S0-S1| (peF = 2*16*(S0-S1))
    c = fd_abs / max(fdcnt_sum, EPS)
    vessel = np.sqrt(N * 512.0 * c ** 2)
    size_t = np.sqrt(float(sum((2.0 ** np.arange(9, 1, -1)) ** 2)))
    loss_fd = vessel / size_t / N

    out = BETA * bce + GAMMA * loss_fd + ALPHA * loss_patch
    return np.asarray(out, dtype=np.float32)


_last_exec_ns = {"ns": None}


def _run(prediction, ground_truth, trace=False, tmpdir=None):
    nc = _build()
    in_maps = _make_in_maps(prediction, ground_truth)
    res = run_bass_kernel_spmd(
        nc, in_maps, list(range(NCORES)), trace=trace, tmpdir=tmpdir
    )
    _last_exec_ns["ns"] = res.exec_time_ns
    return _combine(res.results), res


def kernel(prediction: np.ndarray, ground_truth: np.ndarray) -> np.ndarray:
    out, _ = _run(prediction, ground_truth, trace=False)
    return out


# revision 15
# speedup vs baseline: 1.1441x; 1.1441x over previous
"""Trainium2 Bass kernel for nn_C_Loss_52338471469527 (BCE + fd + patch loss).

Strategy (pure data-parallel over batch, 8 cores x 8 images):
  Device computes, per core, from its (8,512,512) shard of prediction/gt:
    - bce partial sums  Sum ln(sel), sel = g?p:(1-p), via prod=(1-2p)(1-2g)
      and ACT Ln with input affine ln(-0.5*prod+0.5)  [phase 2, ln table set]
    - patch-grid partials: per-pixel |sig-g| = |tanh(p/2)+(1-2g)|/2 accumulated
      per tile-row-position, column-block-64 sums on device
    - fd partials: 4x4-pooled sums of f=th+ug and ug via stride-4 accumulating
      PE matmuls; Sum|pooled| and count(pooled gt>0) accumulated on device
  Host (numpy, float64) does the final tiny nonlinear combine across cores.
"""

import os
import sys

os.environ.setdefault("MYCRO_LOCAL_CACHE", "1")
sys.path.insert(0, "/opt/trn_rl_repo")

from contextlib import ExitStack

import ml_dtypes
import numpy as np

import concourse.bass as bass
import concourse.tile as tile
from concourse import bacc, mybir
from concourse.bass_utils import run_bass_kernel_spmd

F32 = mybir.dt.float32
BF16 = mybir.dt.bfloat16
AFT = mybir.ActivationFunctionType
ALU = mybir.AluOpType

N, H, W = 64, 512, 512
NCORES = 8
NPER = N // NCORES          # images per core
ROWS = NPER * H             # 4096 rows per core
TILE_P, TILE_F = 128, 512
NT = ROWS // TILE_P         # 32 tiles per core
TPOS = H // TILE_P          # 4 tile positions per image
LN_CHUNK = 4                # tiles per phase-2 Ln op
NLN = NT // LN_CHUNK        # 8 Ln ops

BETA, ALPHA, GAMMA = 1.1, 0.03, 0.02
EPS = 1e-6
PATCH_SIZES = (256, 128, 64)


def _device_program(ctx: ExitStack, tc: "tile.TileContext", aps: dict):
    nc = tc.nc
    p_d, g_d = aps["p"], aps["g"]
    p4_d = aps["p4"]
    o_dseg, o_bce, o_fd, o_cnt = aps["o_dseg"], aps["o_bce"], aps["o_fd"], aps["o_cnt"]

    io = ctx.enter_context(tc.tile_pool(name="io", bufs=4))
    work = ctx.enter_context(tc.tile_pool(name="work", bufs=4))
    persist = ctx.enter_context(tc.tile_pool(name="persist", bufs=1))
    pe_pool = ctx.enter_context(tc.tile_pool(name="pe", bufs=2, space="PSUM"))
    uacc_pool = ctx.enter_context(tc.tile_pool(name="uacc", bufs=1, space="PSUM"))

    # Constant pooling matrix P4[k, m] = (k//4 == m)
    P4 = persist.tile([128, 32], BF16, tag="p4c")
    nc.sync.dma_start(P4[:], p4_d)

    # Persistent state
    prod_slab = persist.tile([128, NT * TILE_F], BF16, tag="prod")  # 32KB/part
    acc_slab = persist.tile([128, TPOS * TILE_F], BF16, tag="accd")
    fdabs_cols = persist.tile([32, NT // 2], F32, tag="fdabs")
    fdcnt_cols = persist.tile([32, NT // 2], F32, tag="fdcnt")
    bce_acc = persist.tile([128, NLN], F32, tag="bce")
    half = persist.tile([128, 1], F32, tag="half")
    uacc = [
        uacc_pool.tile([32, 128], F32, tag=f"uacc{t}", name=f"uacc{t}")
        for t in range(TPOS)
    ]

    nc.gpsimd.memset(acc_slab[:], 0.0)
    nc.vector.memset(half[:], 0.5)

    tanh_insts = []
    PF = 2 * TILE_F  # 1024: columns per tile pair

    # ---- Phase 1: stream 16 tile-pairs (tanh table set on ACT) ----
    for pair in range(NT // 2):
        i0 = 2 * pair
        t0 = i0 % TPOS           # in {0, 2}; pair covers t0, t0+1
        img = i0 // TPOS
        first_img = img == 0
        last_img = img == NPER - 1

        pp = io.tile([TILE_P, PF], F32, tag="p")
        nc.sync.dma_start(pp[:, :TILE_F], p_d[i0 * TILE_P:(i0 + 1) * TILE_P, :])
        nc.sync.dma_start(
            pp[:, TILE_F:], p_d[(i0 + 1) * TILE_P:(i0 + 2) * TILE_P, :]
        )
        gp = io.tile([TILE_P, PF], F32, tag="g")
        nc.sync.dma_start(gp[:, :TILE_F], g_d[i0 * TILE_P:(i0 + 1) * TILE_P, :])
        nc.sync.dma_start(
            gp[:, TILE_F:], g_d[(i0 + 1) * TILE_P:(i0 + 2) * TILE_P, :]
        )

        # thug slab: [ th(1024) | ug(1024) ] bf16
        thug = work.tile([TILE_P, 2 * PF], BF16, tag="thug")
        ti = nc.scalar.activation(
            thug[:, :PF], pp[:], AFT.Tanh, bias=0.0, scale=0.5
        )
        tanh_insts.append(ti)
        nc.gpsimd.tensor_scalar(
            thug[:, PF:], gp[:], -2.0, 1.0, op0=ALU.mult, op1=ALU.add
        )
        up = work.tile([TILE_P, PF], BF16, tag="up")
        nc.gpsimd.tensor_scalar(
            up[:], pp[:], -2.0, 1.0, op0=ALU.mult, op1=ALU.add
        )

        # prod = up * ug  -> BCE operand for phase 2
        nc.vector.tensor_mul(
            prod_slab[:, i0 * TILE_F:(i0 + 2) * TILE_F], up[:], thug[:, PF:]
        )
        # m = ug * th = |sig - g|*2 - 1 per pixel; accumulate per t-section
        mp = work.tile([TILE_P, PF], BF16, tag="m")
        nc.vector.tensor_mul(mp[:], thug[:, PF:], thug[:, :PF])
        asec = acc_slab[:, t0 * TILE_F:(t0 + 2) * TILE_F]
        nc.vector.tensor_add(asec, asec, mp[:])

        # PE pooling of the whole thug slab: 4 stride-4 matmuls ->
        # psum [32, 512] = [pool(th0)|pool(th1)|pool(ug0)|pool(ug1)]
        tv = thug[:].rearrange("p (n j) -> p n j", j=4)
        pe = pe_pool.tile([32, 512], F32, tag="pe")
        for j in range(4):
            nc.tensor.matmul(
                pe[:], P4[:], tv[:, :, j], start=(j == 0), stop=(j == 3)
            )
        # fd: tmp = pooled th + pooled ug (= pooled 2e), per 4x4 cell.
        # DVE can read only one PSUM operand, so ACT (next to PSUM) stages
        # the pooled-ug half into SBUF first.
        ug_pool = work.tile([32, 256], F32, tag="ugpool")
        nc.scalar.copy(ug_pool[:], pe[:, 256:512])
        tmp = work.tile([32, 256], F32, tag="fdtmp")
        nc.vector.tensor_add(tmp[:], ug_pool[:], pe[:, 0:256])
        abs_scr = work.tile([32, 256], BF16, tag="abssc")
        nc.scalar.activation(
            abs_scr[:], tmp[:], AFT.Abs, bias=0.0, scale=1.0,
            accum_out=fdabs_cols[:, pair:pair + 1],
        )
        cnt_scr = work.tile([32, 256], F32, tag="cnts")
        nc.vector.tensor_scalar(
            cnt_scr[:], ug_pool[:], 16.0, 0.0, op0=ALU.is_lt, op1=ALU.add,
            accum_out=fdcnt_cols[:, pair:pair + 1],
        )
        # uacc[t] += pooled ug (persistent PSUM accumulation across images)
        for sec in range(2):
            us = thug[:, PF + sec * TILE_F: PF + (sec + 1) * TILE_F]
            uv = us.rearrange("p (n j) -> p n j", j=4)
            for j in range(4):
                nc.tensor.matmul(
                    uacc[t0 + sec][:],
                    P4[:],
                    uv[:, :, j],
                    start=(first_img and j == 0),
                    stop=(last_img and j == 3),
                    skip_group_check=True,
                )

    # ---- Phase 2: BCE ln over the prod slab (ln table set on ACT) ----
    lnw = NT * TILE_F // NLN
    ln_insts = []
    for k in range(NLN):
        lo = work.tile([128, lnw], BF16, tag="lnout")
        li = nc.scalar.activation(
            lo[:],
            prod_slab[:, k * lnw:(k + 1) * lnw],
            AFT.Ln,
            bias=half[:],
            scale=-0.5,
            accum_out=bce_acc[:, k:k + 1],
        )
        ln_insts.append(li)
    # Keep every Ln after every Tanh on ACT: one table switch, not 16.
    if os.environ.get("K_DEP_CHAIN", "1") == "1":
        for ti in tanh_insts:
            tile.add_dep_helper(ln_insts[0].ins, ti.ins, sync=False,
                                reason="act table set ordering")
        for a, b in zip(ln_insts, ln_insts[1:]):
            tile.add_dep_helper(b.ins, a.ins, sync=False,
                                reason="act table set ordering")

    # ---- Final on-device reductions + output DMA ----
    dseg = persist.tile([128, 4 * 8], F32, tag="dseg")
    av = acc_slab[:].rearrange("p (b c) -> p b c", c=64)
    nc.vector.tensor_reduce(
        dseg[:], av, axis=mybir.AxisListType.X, op=ALU.add
    )
    fd_out = persist.tile([32, 2], F32, tag="fdout")
    nc.vector.tensor_reduce(
        fd_out[:, 0:1], fdabs_cols[:], axis=mybir.AxisListType.X, op=ALU.add
    )
    nc.vector.tensor_reduce(
        fd_out[:, 1:2], fdcnt_cols[:], axis=mybir.AxisListType.X, op=ALU.add
    )

    cnt_sb = persist.tile([32, 512], F32, tag="cntsb")
    for t in range(TPOS):
        nc.scalar.copy(cnt_sb[:, t * 128:(t + 1) * 128], uacc[t][:])

    nc.sync.dma_start(o_dseg, dseg[:])
    nc.sync.dma_start(o_bce, bce_acc[:])
    nc.sync.dma_start(o_fd, fd_out[:])
    nc.sync.dma_start(o_cnt, cnt_sb[:])


_built = {}


def _build():
    if "nc" in _built:
        return _built["nc"]
    nc = bacc.Bacc(
        "TRN2", target_bir_lowering=False, debug=False, num_devices=NCORES
    )
    aps = {
        "p": nc.dram_tensor("p", [ROWS, W], F32, kind="ExternalInput").ap(),
        "g": nc.dram_tensor("g", [ROWS, W], F32, kind="ExternalInput").ap(),
        "p4": nc.dram_tensor("p4", [128, 32], BF16, kind="ExternalInput").ap(),
        "o_dseg": nc.dram_tensor("o_dseg", [128, 32], F32, kind="ExternalOutput").ap(),
        "o_bce": nc.dram_tensor("o_bce", [128, NLN], F32, kind="ExternalOutput").ap(),
        "o_fd": nc.dram_tensor("o_fd", [32, 2], F32, kind="ExternalOutput").ap(),
        "o_cnt": nc.dram_tensor("o_cnt", [32, 512], F32, kind="ExternalOutput").ap(),
    }
    with tile.TileContext(nc) as tc:
        with ExitStack() as ctx:
            _device_program(ctx, tc, aps)
    nc.compile()
    _built["nc"] = nc
    return nc


def _p4_matrix():
    m = np.zeros((128, 32), dtype=ml_dtypes.bfloat16)
    m[np.arange(128), np.arange(128) // 4] = 1.0
    return m


def _make_in_maps(prediction: np.ndarray, ground_truth: np.ndarray):
    p = np.ascontiguousarray(
        np.asarray(prediction, dtype=np.float32).reshape(N * H, W)
    )
    g = np.ascontiguousarray(
        np.asarray(ground_truth, dtype=np.float32).reshape(N * H, W)
    )
    p4 = _p4_matrix()
    maps = []
    for c in range(NCORES):
        sl = slice(c * ROWS, (c + 1) * ROWS)
        maps.append({"p": p[sl], "g": g[sl], "p4": p4})
    return maps


def _combine(results: list) -> np.float32:
    """Host-side final combine across the 8 cores (float64)."""
    bce_sum = 0.0
    dsum8 = np.zeros((8, 8), dtype=np.float64)   # per-512x512-grid 64px blocks
    cnt8 = np.zeros((8, 8), dtype=np.float64)
    fdabs_sum = 0.0
    fdcnt_sum = 0.0
    for r in results:
        bce_sum += np.sum(r["o_bce"].astype(np.float64))
        dseg = r["o_dseg"].astype(np.float64)      # [128, 4*8]
        for t in range(TPOS):
            blk = dseg[:, t * 8:(t + 1) * 8]       # [128, 8] sums of m=ug*th
            # sum|f| over 64-row half = sum(m) + 64rows*64cols*NPER
            const = 64.0 * 64.0 * NPER
            dsum8[2 * t, :] += 0.5 * (blk[:64, :].sum(axis=0) + const)
            dsum8[2 * t + 1, :] += 0.5 * (blk[64:, :].sum(axis=0) + const)
        ocnt = r["o_cnt"].astype(np.float64)       # [32, 4*128]
        for t in range(TPOS):
            u = ocnt[:, t * 128:(t + 1) * 128]     # [32, 128] pooled-ug sums
            # u = sum_imgs sum_4x4 ug, ug = 1-2g  =>  sum_imgs sum_4x4 g:
            gc = (16.0 * NPER - u) / 2.0
            for rb in range(2):
                for cb in range(8):
                    cnt8[2 * t + rb, cb] += gc[
                        rb * 16:(rb + 1) * 16, cb * 16:(cb + 1) * 16
                    ].sum()
        ofd = r["o_fd"].astype(np.float64)
        fdabs_sum += ofd[:, 0].sum()
        fdcnt_sum += ofd[:, 1].sum()

    # BCE
    bce = -bce_sum / (N * H * W)

    # Patch loss
    total = 0.0
    for ps in PATCH_SIZES:
        k = ps // 64                      # 64px blocks per patch side
        nh = 8 // k
        d = dsum8.reshape(nh, k, nh, k).sum(axis=(1, 3))
        c = cnt8.reshape(nh, k, nh, k).sum(axis=(1, 3))
        cc = d / np.maximum(c, EPS)
        total_sum = np.sum(float(ps * N) * cc ** 2)
        local = np.clip(np.sqrt(total_sum) / 512.0 / N, 0.0, 1.0)
        total += local
    loss_patch = total / 64.0

    # fd loss
    fd_abs = fdabs_sum / 32.0             # sum |S0-S1| (peF = 2*16*(S0-S1))
    c = fd_abs / max(fdcnt_sum, EPS)
    vessel = np.sqrt(N * 512.0 * c ** 2)
    size_t = np.sqrt(float(sum((2.0 ** np.arange(9, 1, -1)) ** 2)))
    loss_fd = vessel / size_t / N

    out = BETA * bce + GAMMA * loss_fd + ALPHA * loss_patch
    return np.asarray(out, dtype=np.float32)


_last_exec_ns = {"ns": None}


def _run(prediction, ground_truth, trace=False, tmpdir=None):
    nc = _build()
    in_maps = _make_in_maps(prediction, ground_truth)
    res = run_bass_kernel_spmd(
        nc, in_maps, list(range(NCORES)), trace=trace, tmpdir=tmpdir
    )
    _last_exec_ns["ns"] = res.exec_time_ns
    return _combine(res.results), res


def kernel(prediction: np.ndarray, ground_truth: np.ndarray) -> np.ndarray:
    out, _ = _run(prediction, ground_truth, trace=False)
    return out


# revision 17
# speedup vs baseline: 1.2134x; 1.0606x over previous
"""Trainium2 Bass kernel for nn_C_Loss_52338471469527 (BCE + fd + patch loss).

Strategy (pure data-parallel over batch, 8 cores x 8 images):
  Device computes, per core, from its (8,512,512) shard of prediction/gt:
    - bce partial sums  Sum ln(sel), sel = g?p:(1-p), via prod=(1-2p)(1-2g)
      and ACT Ln with input affine ln(-0.5*prod+0.5)  [phase 2, ln table set]
    - patch-grid partials: per-pixel |sig-g| = |tanh(p/2)+(1-2g)|/2 accumulated
      per tile-row-position, column-block-64 sums on device
    - fd partials: 4x4-pooled sums of f=th+ug and ug via stride-4 accumulating
      PE matmuls; Sum|pooled| and count(pooled gt>0) accumulated on device
  Host (numpy, float64) does the final tiny nonlinear combine across cores.
"""

import os
import sys

os.environ.setdefault("MYCRO_LOCAL_CACHE", "1")
sys.path.insert(0, "/opt/trn_rl_repo")

from contextlib import ExitStack

import ml_dtypes
import numpy as np

import concourse.bass as bass
import concourse.tile as tile
from concourse import bacc, mybir
from concourse.bass_utils import run_bass_kernel_spmd

F32 = mybir.dt.float32
BF16 = mybir.dt.bfloat16
AFT = mybir.ActivationFunctionType
ALU = mybir.AluOpType

N, H, W = 64, 512, 512
NCORES = 8
NPER = N // NCORES          # images per core
ROWS = NPER * H             # 4096 rows per core
TILE_P, TILE_F = 128, 512
NT = ROWS // TILE_P         # 32 tiles per core
TPOS = H // TILE_P          # 4 tile positions per image
LN_CHUNK = 4                # tiles per phase-2 Ln op
NLN = NT // LN_CHUNK        # 8 Ln ops

BETA, ALPHA, GAMMA = 1.1, 0.03, 0.02
EPS = 1e-6
PATCH_SIZES = (256, 128, 64)


def _device_program(ctx: ExitStack, tc: "tile.TileContext", aps: dict):
    nc = tc.nc
    p_d, g_d = aps["p"], aps["g"]
    p4_d = aps["p4"]
    o_dseg, o_bce, o_fd, o_cnt = aps["o_dseg"], aps["o_bce"], aps["o_fd"], aps["o_cnt"]

    io = ctx.enter_context(tc.tile_pool(name="io", bufs=4))
    work = ctx.enter_context(tc.tile_pool(name="work", bufs=4))
    persist = ctx.enter_context(tc.tile_pool(name="persist", bufs=1))
    pe_pool = ctx.enter_context(tc.tile_pool(name="pe", bufs=3, space="PSUM"))

    # Constant pooling matrix P4[k, m] = (k//4 == m)
    P4 = persist.tile([128, 32], BF16, tag="p4c")
    nc.sync.dma_start(P4[:], p4_d)

    # Persistent state
    prod_slab = persist.tile([128, NT * TILE_F], BF16, tag="prod")  # 32KB/part
    acc_slab = persist.tile([128, TPOS * TILE_F], BF16, tag="accd")
    fdabs_cols = persist.tile([32, NT // 2], F32, tag="fdabs")
    fdcnt_cols = persist.tile([32, NT // 2], F32, tag="fdcnt")
    bce_acc = persist.tile([128, NLN], F32, tag="bce")
    half = persist.tile([128, 1], F32, tag="half")
    cnt_slab = persist.tile([32, (NT // 2) * 256], F32, tag="cntslab")

    nc.gpsimd.memset(acc_slab[:], 0.0)
    nc.vector.memset(half[:], 0.5)

    tanh_insts = []
    ln_insts = []
    lnw = NT * TILE_F // NLN

    def emit_ln(k):
        lo = work.tile([128, lnw], BF16, tag="lnout", name=f"lnout{k}")
        li = nc.scalar.activation(
            lo[:],
            prod_slab[:, k * lnw:(k + 1) * lnw],
            AFT.Ln,
            bias=half[:],
            scale=-0.5,
            accum_out=bce_acc[:, k:k + 1],
        )
        # after every tanh emitted so far, and after the previous ln
        if os.environ.get("K_DEP_CHAIN", "1") == "1":
            for ti in tanh_insts:
                tile.add_dep_helper(li.ins, ti.ins, sync=False,
                                    reason="act table set ordering")
            if ln_insts:
                tile.add_dep_helper(li.ins, ln_insts[-1].ins, sync=False,
                                    reason="act table set ordering")
        ln_insts.append(li)

    PF = 2 * TILE_F  # 1024: columns per tile pair

    # ---- Phase 1: stream 16 tile-pairs (tanh table set on ACT) ----
    for pair in range(NT // 2):
        i0 = 2 * pair
        t0 = i0 % TPOS           # in {0, 2}; pair covers t0, t0+1
        img = i0 // TPOS
        first_img = img == 0
        last_img = img == NPER - 1

        pp = io.tile([TILE_P, PF], F32, tag="p")
        nc.sync.dma_start(pp[:, :TILE_F], p_d[i0 * TILE_P:(i0 + 1) * TILE_P, :])
        nc.sync.dma_start(
            pp[:, TILE_F:], p_d[(i0 + 1) * TILE_P:(i0 + 2) * TILE_P, :]
        )
        gp = io.tile([TILE_P, PF], F32, tag="g")
        nc.sync.dma_start(gp[:, :TILE_F], g_d[i0 * TILE_P:(i0 + 1) * TILE_P, :])
        nc.sync.dma_start(
            gp[:, TILE_F:], g_d[(i0 + 1) * TILE_P:(i0 + 2) * TILE_P, :]
        )

        # thug slab: [ th(1024) | ug(1024) ] bf16
        thug = work.tile([TILE_P, 2 * PF], BF16, tag="thug")
        ti = nc.scalar.activation(
            thug[:, :PF], pp[:], AFT.Tanh, bias=0.0, scale=0.5
        )
        tanh_insts.append(ti)
        nc.gpsimd.tensor_scalar(
            thug[:, PF:], gp[:], -2.0, 1.0, op0=ALU.mult, op1=ALU.add
        )
        up = work.tile([TILE_P, PF], BF16, tag="up")
        nc.gpsimd.tensor_scalar(
            up[:], pp[:], -2.0, 1.0, op0=ALU.mult, op1=ALU.add
        )

        # prod = up * ug  -> BCE operand for phase 2
        nc.vector.tensor_mul(
            prod_slab[:, i0 * TILE_F:(i0 + 2) * TILE_F], up[:], thug[:, PF:]
        )
        # m = ug * th = |sig - g|*2 - 1 per pixel; accumulate per t-section
        mp = work.tile([TILE_P, PF], BF16, tag="m")
        nc.vector.tensor_mul(mp[:], thug[:, PF:], thug[:, :PF])
        asec = acc_slab[:, t0 * TILE_F:(t0 + 2) * TILE_F]
        nc.vector.tensor_add(asec, asec, mp[:])

        # PE pooling of the whole thug slab: 4 stride-4 matmuls ->
        # psum [32, 512] = [pool(th0)|pool(th1)|pool(ug0)|pool(ug1)]
        tv = thug[:].rearrange("p (n j) -> p n j", j=4)
        pe = pe_pool.tile([32, 512], F32, tag="pe")
        for j in range(4):
            nc.tensor.matmul(
                pe[:], P4[:], tv[:, :, j], start=(j == 0), stop=(j == 3)
            )
        # fd: tmp = pooled th + pooled ug (= pooled 2e), per 4x4 cell.
        # DVE can read only one PSUM operand, so ACT (next to PSUM) stages
        # the pooled-ug half into SBUF first.
        ug_pool = cnt_slab[:, pair * 256:(pair + 1) * 256]
        nc.scalar.copy(ug_pool, pe[:, 256:512])
        tmp = work.tile([32, 256], F32, tag="fdtmp")
        nc.vector.tensor_add(tmp[:], ug_pool, pe[:, 0:256])
        abs_scr = work.tile([32, 256], BF16, tag="abssc")
        nc.scalar.activation(
            abs_scr[:], tmp[:], AFT.Abs, bias=0.0, scale=1.0,
            accum_out=fdabs_cols[:, pair:pair + 1],
        )
        cnt_scr = work.tile([32, 256], F32, tag="cnts")
        nc.vector.tensor_scalar(
            cnt_scr[:], ug_pool, 16.0, 0.0, op0=ALU.is_lt, op1=ALU.add,
            accum_out=fdcnt_cols[:, pair:pair + 1],
        )
        if pair == NT // 2 - 1 - 4:
            # first half of the BCE Ln work: inputs (pairs 0..7) are done,
            # and it overlaps the remaining phase-1 streaming
            for k in range(NLN // 2):
                emit_ln(k)

    # ---- Phase 2: remaining BCE Ln work ----
    for k in range(NLN // 2, NLN):
        emit_ln(k)

    # ---- Final on-device reductions + output DMA ----
    dseg = persist.tile([128, 4 * 8], F32, tag="dseg")
    av = acc_slab[:].rearrange("p (b c) -> p b c", c=64)
    nc.vector.tensor_reduce(
        dseg[:], av, axis=mybir.AxisListType.X, op=ALU.add
    )
    fd_out = persist.tile([32, 2], F32, tag="fdout")
    nc.vector.tensor_reduce(
        fd_out[:, 0:1], fdabs_cols[:], axis=mybir.AxisListType.X, op=ALU.add
    )
    nc.vector.tensor_reduce(
        fd_out[:, 1:2], fdcnt_cols[:], axis=mybir.AxisListType.X, op=ALU.add
    )

    nc.sync.dma_start(o_dseg, dseg[:])
    nc.sync.dma_start(o_bce, bce_acc[:])
    nc.sync.dma_start(o_fd, fd_out[:])
    nc.sync.dma_start(o_cnt, cnt_slab[:])


_built = {}


def _build():
    if "nc" in _built:
        return _built["nc"]
    nc = bacc.Bacc(
        "TRN2", target_bir_lowering=False, debug=False, num_devices=NCORES
    )
    aps = {
        "p": nc.dram_tensor("p", [ROWS, W], F32, kind="ExternalInput").ap(),
        "g": nc.dram_tensor("g", [ROWS, W], F32, kind="ExternalInput").ap(),
        "p4": nc.dram_tensor("p4", [128, 32], BF16, kind="ExternalInput").ap(),
        "o_dseg": nc.dram_tensor("o_dseg", [128, 32], F32, kind="ExternalOutput").ap(),
        "o_bce": nc.dram_tensor("o_bce", [128, NLN], F32, kind="ExternalOutput").ap(),
        "o_fd": nc.dram_tensor("o_fd", [32, 2], F32, kind="ExternalOutput").ap(),
        "o_cnt": nc.dram_tensor("o_cnt", [32, (NT // 2) * 256], F32, kind="ExternalOutput").ap(),
    }
    with tile.TileContext(nc) as tc:
        with ExitStack() as ctx:
            _device_program(ctx, tc, aps)
    nc.compile()
    _built["nc"] = nc
    return nc


def _p4_matrix():
    m = np.zeros((128, 32), dtype=ml_dtypes.bfloat16)
    m[np.arange(128), np.arange(128) // 4] = 1.0
    return m


def _make_in_maps(prediction: np.ndarray, ground_truth: np.ndarray):
    p = np.ascontiguousarray(
        np.asarray(prediction, dtype=np.float32).reshape(N * H, W)
    )
    g = np.ascontiguousarray(
        np.asarray(ground_truth, dtype=np.float32).reshape(N * H, W)
    )
    p4 = _p4_matrix()
    maps = []
    for c in range(NCORES):
        sl = slice(c * ROWS, (c + 1) * ROWS)
        maps.append({"p": p[sl], "g": g[sl], "p4": p4})
    return maps


def _combine(results: list) -> np.float32:
    """Host-side final combine across the 8 cores (float64)."""
    bce_sum = 0.0
    dsum8 = np.zeros((8, 8), dtype=np.float64)   # per-512x512-grid 64px blocks
    cnt8 = np.zeros((8, 8), dtype=np.float64)
    fdabs_sum = 0.0
    fdcnt_sum = 0.0
    for r in results:
        bce_sum += np.sum(r["o_bce"].astype(np.float64))
        dseg = r["o_dseg"].astype(np.float64)      # [128, 4*8]
        for t in range(TPOS):
            blk = dseg[:, t * 8:(t + 1) * 8]       # [128, 8] sums of m=ug*th
            # sum|f| over 64-row half = sum(m) + 64rows*64cols*NPER
            const = 64.0 * 64.0 * NPER
            dsum8[2 * t, :] += 0.5 * (blk[:64, :].sum(axis=0) + const)
            dsum8[2 * t + 1, :] += 0.5 * (blk[64:, :].sum(axis=0) + const)
        ocnt = r["o_cnt"].astype(np.float64)       # [32, 16*256]
        for pair in range(NT // 2):
            t0 = (2 * pair) % TPOS
            sec = ocnt[:, pair * 256:(pair + 1) * 256]
            for s in range(2):
                u = sec[:, s * 128:(s + 1) * 128]  # [32,128] pooled ug, 1 tile
                gc = (16.0 - u) / 2.0              # per-cell sum of g
                t = t0 + s
                for rb in range(2):
                    for cb in range(8):
                        cnt8[2 * t + rb, cb] += gc[
                            rb * 16:(rb + 1) * 16, cb * 16:(cb + 1) * 16
                        ].sum()
        ofd = r["o_fd"].astype(np.float64)
        fdabs_sum += ofd[:, 0].sum()
        fdcnt_sum += ofd[:, 1].sum()

    # BCE
    bce = -bce_sum / (N * H * W)

    # Patch loss
    total = 0.0
    for ps in PATCH_SIZES:
        k = ps // 64                      # 64px blocks per patch side
        nh = 8 // k
        d = dsum8.reshape(nh, k, nh, k).sum(axis=(1, 3))
        c = cnt8.reshape(nh, k, nh, k).sum(axis=(1, 3))
        cc = d / np.maximum(c, EPS)
        total_sum = np.sum(float(ps * N) * cc ** 2)
        local = np.clip(np.sqrt(total_sum) / 512.0 / N, 0.0, 1.0)
        total += local
    loss_patch = total / 64.0

    # fd loss
    fd_abs = fdabs_sum / 32.0             # sum |S0-S1| (peF = 2*16*(S0-S1))
    c = fd_abs / max(fdcnt_sum, EPS)
    vessel = np.sqrt(N * 512.0 * c ** 2)
    size_t = np.sqrt(float(sum((2.0 ** np.arange(9, 1, -1)) ** 2)))
    loss_fd = vessel / size_t / N

    out = BETA * bce + GAMMA * loss_fd + ALPHA * loss_patch
    return np.asarray(out, dtype=np.float32)


_last_exec_ns = {"ns": None}


def _run(prediction, ground_truth, trace=False, tmpdir=None):
    nc = _build()
    in_maps = _make_in_maps(prediction, ground_truth)
    res = run_bass_kernel_spmd(
        nc, in_maps, list(range(NCORES)), trace=trace, tmpdir=tmpdir
    )
    _last_exec_ns["ns"] = res.exec_time_ns
    return _combine(res.results), res


def kernel(prediction: np.ndarray, ground_truth: np.ndarray) -> np.ndarray:
    out, _ = _run(prediction, ground_truth, trace=False)
    return out
